# revision 4
# baseline (speedup 1.0000x reference)
"""Trainium2 Bass kernel for nn_ExpansionContrastModule.

Sharding: 8 cores = 4 batches x 2 H-halves (80 rows each). Bottom halves are
row-flipped on the host (conv weights H-flipped to match) so that image-pad
rows always sit at shard-top; the dwconv-product-sort stage is flip-invariant
because it only depends on the +/-v_m offset pairs and the sort is
permutation-invariant over directions.

Within a core the 80 owned rows split into two 40-row sub-halves A/B mapped to
SBUF partitions 0:64 / 64:128 (64 channels each), computed in lockstep:
conv matmuls use block-diagonal duplicated weights (K=128, M=128) and all
vector ops run fully packed [128, N].

Restructured dwconv-contrast: with o1_m(p) = x(p) - x(p + v_m),
  o_m(p) = o1_m(p) * (x(p) - x(p - v_m)) = -o1_m(p) * o1_m(p - v_m),
so only 4 difference maps per branch are needed; the negation is absorbed by
using adjusted scales s'[g, j] = -scales3[g, 3-j] on the sorted products
t_m = o1_m(p) * o1_m(p - v_m).

Geometry (per half, local coords): shard = 100 rows (10 pad/halo + 80 + 10
halo). Owned rows = shard 10..89; A owns 10..49, B owns 50..89.
x0 (in_conv out): 60 rows per half, row r <-> shard r (A) / 40+r (B).
x_k (branch conv out): rows_x = 40+2d rows, row i <-> shard 10-d+i (A),
50-d+i (B); width Wx = 160+4d with real cols at [2d, 2d+160).
Shard rows 0..9 are always image-pad (flip trick) -> x0 rows 0..9 and
x_k rows 0..d-1 of the A half are memset to zero.

v2 changes vs baseline:
- all DMAs on HWDGE (nc.sync) instead of SWDGE (gpsimd) - SWDGE descriptor
  generation starves while DVE holds the shared SBUF port.
- phase A loads cen in big chunks instead of per-row-tile.
- final stage computes the z-chain once (it is shared between the two mask
  halves; baseline computed it twice) at 20-row granularity, with bf16
  outputs (host upcasts; tolerance is 2e-2, bf16 rounding adds ~4e-4).
"""

import os

os.environ.setdefault("MYCRO_LOCAL_CACHE", "1")

import numpy as np
import ml_dtypes

import concourse.bass as bass
import concourse.bacc as bacc
import concourse.mybir as mybir
from concourse.tile import TileContext
from concourse import bass_utils

W = 160
SH = 100          # shard rows
HALO = 10
OWNH = 40         # owned rows per half
C = 256
CH = 64           # trunk channels
RB = 10           # contrast-stage block rows
KS = [1, 3, 5, 7]         # branch conv kernel sizes
DIL = [1, 3, 5, 7]        # branch dwconv dilations (= shift)
TAP_OFF = [0, 1, 10, 35]  # cumulative tap offsets into wtap
NTAP = 84
X0R = 60
WP0 = W + 6       # x0 width, real cols at [3, 163)
RT = 3            # conv rows per psum tile (3*160 = 480 <= 512)
FCH = 20          # final-stage chunk rows

F32 = mybir.dt.float32
BF16 = mybir.dt.bfloat16
ALU = mybir.AluOpType
ACTF = mybir.ActivationFunctionType


def build_nc():
    nc = bacc.Bacc("TRN2", target_bir_lowering=False, debug=False,
                   enable_asserts=False, num_devices=8)

    def dram(name, shape, dt, kind="ExternalInput"):
        return nc.dram_tensor(name, list(shape), dt, kind=kind).ap()

    cenb = [dram(f"cenb{c}", (128, SH * W), BF16) for c in range(2)]
    win = dram("win", (128, 128), BF16)
    wtap = dram("wtap", (128, NTAP * 128), BF16)
    wbc = dram("wbc", (128, 128), BF16)
    wfc = dram("wfc", (128, 2), BF16)
    ones1 = dram("ones1", (1, 128), BF16)
    bin_ = dram("bin", (128, 1), F32)
    cb = dram("cb", (128, 4), F32)
    sadj = dram("sadj", (128, 16), F32)
    bnsc = dram("bnsc", (128, 1), F32)
    bnbi = dram("bnbi", (128, 1), F32)
    fcb = dram("fcb", (1, 1), F32)
    outs = [dram(f"out{c}", (128, OWNH * 2 * W), BF16, kind="ExternalOutput")
            for c in range(2)]

    with TileContext(nc) as tc:
        with tc.tile_pool(name="cpool", bufs=1) as cp, \
             tc.tile_pool(name="inpool", bufs=2) as ip, \
             tc.tile_pool(name="x0pool", bufs=1) as x0p, \
             tc.tile_pool(name="xpool", bufs=2) as xp, \
             tc.tile_pool(name="o1pool", bufs=1) as o1p, \
             tc.tile_pool(name="tpool", bufs=1) as tp, \
             tc.tile_pool(name="ypool", bufs=2) as yp, \
             tc.tile_pool(name="vpool", bufs=1) as vp, \
             tc.tile_pool(name="pspool", bufs=1, space="PSUM") as pp:

            # ---- constants to SBUF
            win_s = cp.tile_from(win, name="win_s")
            wtap_s = cp.tile_from(wtap, name="wtap_s")
            wbc_s = cp.tile_from(wbc, name="wbc_s")
            wfc_s = cp.tile_from(wfc, name="wfc_s")
            ones_s = cp.tile_from(ones1, name="ones_s")
            bin_s = cp.tile_from(bin_, name="bin_s")
            cb_s = cp.tile_from(cb, name="cb_s")
            sadj_s = cp.tile_from(sadj, name="sadj_s")
            bnsc_s = cp.tile_from(bnsc, name="bnsc_s")
            bnbi_s = cp.tile_from(bnbi, name="bnbi_s")
            fcb_s = cp.tile_from(fcb, name="fcb_s")

            # ---- Phase A: in_conv -> x0 [128, 60*166] bf16
            # cen chunks: per (h, chunk of 15 x0-rows, c) load [128, 15*160].
            CKR = 15
            x0 = x0p.tile([128, X0R * WP0], BF16, name="x0")
            nc.gpsimd.memset(x0[:, :], 0.0)
            for h, base in ((0, 0), (1, 40)):
                for ck in range(0, X0R, CKR):
                    cts = []
                    for c in range(2):
                        ct = ip.tile([128, FCH * W], BF16, tag=f"cin{c}",
                                     name=f"ct{c}")
                        nc.sync.dma_start(
                            out=ct[:, 0:CKR * W],
                            in_=cenb[c][:, (base + ck) * W:(base + ck + CKR) * W])
                        cts.append(ct)
                    for t0 in range(ck, ck + CKR, RT):
                        n = RT * W
                        ps = pp.tile([128, n], F32, tag="cvps", bufs=3,
                                     name="ps_in")
                        for c in range(2):
                            nc.tensor.matmul(
                                ps[h * 64:h * 64 + 64, :],
                                lhsT=win_s[:, c * 64:c * 64 + 64],
                                rhs=cts[c][:, (t0 - ck) * W:(t0 - ck) * W + n],
                                start=(c == 0), stop=(c == 1))
                        nc.scalar.activation(
                            x0[h * 64:h * 64 + 64, :]
                            .rearrange("p (r w) -> p r w", w=WP0)
                            [:, t0:t0 + RT, 3:3 + W],
                            ps[h * 64:h * 64 + 64, :]
                            .rearrange("p (r w) -> p r w", w=W),
                            ACTF.Identity, bias=bin_s[h * 64:h * 64 + 64, 0:1])
            # zero image-pad rows of A half (shard rows 0..9)
            nc.gpsimd.memset(x0[0:64, 0:HALO * WP0], 0.0)

            # ---- vmax / vsum accumulators [128, 40*160] bf16
            vmax = vp.tile([128, OWNH * W], BF16, name="vmax")
            vsum = vp.tile([128, OWNH * W], BF16, name="vsum")

            x0v = x0[:, :].rearrange("p (r w) -> p r w", w=WP0)

            for k in range(4):
                d = DIL[k]
                ksz = KS[k]
                pad = ksz // 2
                rows_x = OWNH + 2 * d
                Wx = W + 4 * d
                xk = xp.tile([128, rows_x * Wx], BF16, tag="x", name=f"x{k}")
                xv = xk[:, :].rearrange("p (r w) -> p r w", w=Wx)
                # zero col pads: strided [rows, 4d] at col 2d+W covers right
                # pad of each row + left pad of next; plus row0 left pad.
                nc.gpsimd.memset(xk[:, 0:2 * d], 0.0)
                nc.gpsimd.memset(
                    xk[:, 2 * d + W:2 * d + W + (rows_x - 1) * Wx]
                    .rearrange("p (r w) -> p r w", w=Wx)[:, :, 0:4 * d], 0.0)
                nc.gpsimd.memset(
                    xk[:, (rows_x - 1) * Wx + 2 * d + W:rows_x * Wx], 0.0)

                # branch conv: psum tiles of RT rows
                for rt in range(0, rows_x, RT):
                    nr = min(RT, rows_x - rt)
                    n = nr * W
                    ps = pp.tile([128, n], F32, tag="cvps", bufs=3,
                                 name=f"ps{k}")
                    ti = 0
                    for ki in range(ksz):
                        for kj in range(ksz):
                            dy, dx = ki - pad, kj - pad
                            tap = TAP_OFF[k] + ki * ksz + kj
                            r0 = HALO - d + rt + dy
                            nc.tensor.matmul(
                                ps[:, :],
                                lhsT=wtap_s[:, tap * 128:tap * 128 + 128],
                                rhs=x0v[:, r0:r0 + nr, 3 + dx:3 + dx + W],
                                start=(ti == 0), stop=(ti == ksz * ksz - 1))
                            ti += 1
                    nc.scalar.activation(
                        xv[:, rt:rt + nr, 2 * d:2 * d + W],
                        ps[:, :].rearrange("p (r w) -> p r w", w=W),
                        ACTF.Identity, bias=cb_s[:, k:k + 1])
                # zero image-pad rows of A half: x rows 0..d-1
                nc.gpsimd.memset(xk[0:64, 0:d * Wx], 0.0)

                # ---- contrast stage: blocks of RB owned rows
                WPK = W + 2 * d
                vs = [(-d, -d), (-d, 0), (-d, d), (0, -d)]
                for b in range(0, OWNH, RB):
                    ts = []
                    for j, (dy, dx) in enumerate(vs):
                        o1 = o1p.tile([128, (RB + d) * WPK], BF16, tag=f"o1_{j}",
                                      name=f"o1_{k}_{b}_{j}")
                        o1v = o1[:, :].rearrange("p (r w) -> p r w", w=WPK)
                        xr0 = d + b   # x row of o1 row 0
                        nc.vector.tensor_sub(
                            o1v[:, :, :],
                            xv[:, xr0:xr0 + RB + d, d:d + WPK],
                            xv[:, xr0 + dy:xr0 + dy + RB + d,
                               d + dx:d + dx + WPK])
                        tj = tp.tile([128, RB * W], BF16, tag=f"t{j}",
                                     name=f"t{k}_{b}_{j}")
                        nc.vector.tensor_mul(
                            tj[:, :].rearrange("p (r w) -> p r w", w=W),
                            o1v[:, 0:RB, d:d + W],
                            o1v[:, -dy:-dy + RB, d - dx:d - dx + W])
                        ts.append(tj)
                    t0_, t1_, t2_, t3_ = [t[:, :] for t in ts]
                    e1 = tp.tile([128, RB * W], BF16, tag="e1",
                                 name=f"e{k}_{b}")[:, :]
                    # 5-comparator sort network (ascending finals:
                    # t3_=o(1), t1_=o(2), t0_=o(3), e1=o(4))
                    nc.vector.tensor_tensor(e1, t0_, t1_, ALU.max)
                    nc.vector.tensor_tensor(t0_, t0_, t1_, ALU.min)
                    nc.vector.tensor_tensor(t1_, t2_, t3_, ALU.max)
                    nc.vector.tensor_tensor(t2_, t2_, t3_, ALU.min)
                    nc.vector.tensor_tensor(t3_, t0_, t2_, ALU.min)
                    nc.vector.tensor_tensor(t0_, t0_, t2_, ALU.max)
                    nc.vector.tensor_tensor(t2_, e1, t1_, ALU.min)
                    nc.vector.tensor_tensor(e1, e1, t1_, ALU.max)
                    nc.vector.tensor_tensor(t1_, t0_, t2_, ALU.min)
                    nc.vector.tensor_tensor(t0_, t0_, t2_, ALU.max)
                    # weighted sum with adjusted scales
                    y = yp.tile([128, RB * W], BF16, tag="y",
                                name=f"y{k}_{b}")[:, :]
                    nc.vector.tensor_scalar_mul(y, t3_, sadj_s[:, 4 * k:4 * k + 1])
                    nc.vector.scalar_tensor_tensor(
                        y, t1_, sadj_s[:, 4 * k + 1:4 * k + 2], y,
                        op0=ALU.mult, op1=ALU.add)
                    nc.vector.scalar_tensor_tensor(
                        y, t0_, sadj_s[:, 4 * k + 2:4 * k + 3], y,
                        op0=ALU.mult, op1=ALU.add)
                    nc.vector.scalar_tensor_tensor(
                        y, e1, sadj_s[:, 4 * k + 3:4 * k + 4], y,
                        op0=ALU.mult, op1=ALU.add)
                    vmx = vmax[:, b * W:(b + RB) * W]
                    vsm = vsum[:, b * W:(b + RB) * W]
                    if k == 0:
                        nc.vector.tensor_copy(vmx, y)
                        nc.vector.tensor_copy(vsm, y)
                    else:
                        nc.vector.tensor_tensor(vmx, vmx, y, ALU.max)
                        nc.vector.tensor_tensor(vsm, vsm, y, ALU.add)

            # ---- final stage: chunks of FCH rows (columns shared by both
            # halves; z-chain computed once, mask extracted per half).
            for f0 in range(0, OWNH, FCH):
                n = FCH * W                       # 3200
                sl = slice(f0 * W, (f0 + FCH) * W)
                mt = o1p.tile([128, n], BF16, tag="o1_0", name="mt")[:, :]
                nc.vector.scalar_tensor_tensor(
                    mt, vsum[:, sl], 0.25, vmax[:, sl],
                    op0=ALU.mult, op1=ALU.add)
                mr = o1p.tile([128, n], BF16, tag="o1_1", name="mr")[:, :]
                nc.scalar.activation(mr, mt, ACTF.Relu)
                zlin = o1p.tile([128, n], BF16, tag="o1_2", name="zlin")[:, :]
                zsig = o1p.tile([128, n], BF16, tag="o1_3", name="zsig")[:, :]
                NSUB = (n + 479) // 480
                for s in range(NSUB):
                    c0, c1 = s * 480, min(n, s * 480 + 480)
                    zps = pp.tile([128, 480], F32, tag="zps", bufs=2,
                                  name="zps")
                    nc.tensor.matmul(zps[:, 0:c1 - c0], lhsT=wbc_s[:, :],
                                     rhs=mr[:, c0:c1], start=True, stop=True)
                    nc.scalar.activation(zlin[:, c0:c1], zps[:, 0:c1 - c0],
                                         ACTF.Identity, bias=bnbi_s[:, 0:1],
                                         scale=bnsc_s[:, 0:1])
                    nc.scalar.activation(zsig[:, c0:c1], zps[:, 0:c1 - c0],
                                         ACTF.Sigmoid, bias=bnbi_s[:, 0:1],
                                         scale=bnsc_s[:, 0:1])
                zt = tp.tile([128, n], BF16, tag="t0", bufs=1, name="zt")[:, :]
                nc.vector.tensor_mul(zt, zlin, zsig)
                for h in range(2):
                    msk = tp.tile([1, n], BF16, tag="msk", bufs=1,
                                  name="msk")
                    mbs = tp.tile([128, n], BF16, tag="mbs", bufs=1,
                                  name="mbs")[:, :]
                    for s in range(NSUB):
                        c0, c1 = s * 480, min(n, s * 480 + 480)
                        lps = pp.tile([1, 480], F32, tag="lps", bufs=1,
                                      name="lps")
                        nc.tensor.matmul(lps[:, 0:c1 - c0],
                                         lhsT=wfc_s[:, h:h + 1],
                                         rhs=zt[:, c0:c1],
                                         start=True, stop=True)
                        nc.scalar.activation(msk[:, c0:c1], lps[:, 0:c1 - c0],
                                             ACTF.Sigmoid, bias=fcb_s[0:1, 0:1])
                        mb = pp.tile([128, 480], F32, tag="mb", bufs=1,
                                     name="mb")
                        nc.tensor.matmul(mb[:, 0:c1 - c0], lhsT=ones_s[:, :],
                                         rhs=msk[:, c0:c1], start=True,
                                         stop=True)
                        nc.scalar.activation(mbs[:, c0:c1], mb[:, 0:c1 - c0],
                                             ACTF.Identity)
                    for c in range(2):
                        cent = ip.tile([128, FCH * W], BF16, tag=f"cin{c}",
                                       name="cent")
                        src = (HALO + h * OWNH + f0) * W
                        nc.sync.dma_start(out=cent[:, :],
                                          in_=cenb[c][:, src:src + n])
                        ot = yp.tile([128, n], BF16, tag=f"ot{c}", bufs=1,
                                     name="ot")
                        nc.vector.scalar_tensor_tensor(
                            ot[:, :], mbs, 1.0, cent[:, :],
                            op0=ALU.add, op1=ALU.mult)
                        dst = (h * OWNH + f0) * W
                        nc.sync.dma_start(out=outs[c][:, dst:dst + n],
                                          in_=ot[:, :])
    nc.compile()
    nc.finalize()
    return nc


_NC_CACHE = None


def _get_nc():
    global _NC_CACHE
    if _NC_CACHE is None:
        _NC_CACHE = build_nc()
    return _NC_CACHE


def _prep_core_inputs(cen_b, flip, wts):
    """cen_b: (256, 160, 160) fp32 for this batch; flip: bottom half?"""
    (w_in, b_in, convs, scales_adj, bc_w, bn_scale, bn_bias,
     fc_w, fc_b) = wts
    lo = (1 if flip else 0) * 80 - HALO
    sh = np.zeros((C, SH, W), np.float32)
    r0, r1 = max(0, lo), min(160, lo + SH)
    sh[:, r0 - lo:r1 - lo] = cen_b[:, r0:r1]
    if flip:
        sh = sh[:, ::-1]
    sh = np.ascontiguousarray(sh)

    bf = ml_dtypes.bfloat16
    wtap = np.zeros((128, NTAP * 128), bf)
    for k in range(4):
        ksz = KS[k]
        cw = convs[k][0]
        if flip:
            cw = cw[:, :, ::-1, :]
        for ki in range(ksz):
            for kj in range(ksz):
                t = TAP_OFF[k] + ki * ksz + kj
                blk = cw[:, :, ki, kj].T.astype(bf)  # [ci, co]
                wtap[0:64, t * 128:t * 128 + 64] = blk
                wtap[64:128, t * 128 + 64:t * 128 + 128] = blk

    win = np.zeros((128, 128), bf)
    win[:, 0:64] = w_in[:, 0:128].T.astype(bf)
    win[:, 64:128] = w_in[:, 128:256].T.astype(bf)

    wbc = np.zeros((128, 128), bf)
    wbc[0:64, 0:64] = bc_w.T.astype(bf)
    wbc[64:128, 64:128] = bc_w.T.astype(bf)

    wfc = np.zeros((128, 2), bf)
    wfc[0:64, 0] = fc_w.astype(bf)
    wfc[64:128, 1] = fc_w.astype(bf)

    dup = lambda v: np.concatenate([v, v]).astype(np.float32).reshape(128, -1)
    m = {
        "cenb0": sh[0:128].reshape(128, SH * W).astype(bf),
        "cenb1": sh[128:256].reshape(128, SH * W).astype(bf),
        "win": win,
        "wtap": wtap,
        "wbc": wbc,
        "wfc": wfc,
        "ones1": np.ones((1, 128), bf),
        "bin": dup(b_in),
        "cb": np.concatenate([np.stack([cb for _, cb in convs], 1)] * 2, 0)
              .astype(np.float32),
        "sadj": np.concatenate([scales_adj.reshape(64, 16)] * 2, 0)
                .astype(np.float32),
        "bnsc": dup(bn_scale),
        "bnbi": dup(bn_bias),
        "fcb": np.full((1, 1), fc_b, np.float32),
    }
    return m


def make_in_maps(inputs):
    cen = np.asarray(inputs["cen"], np.float32)
    w_in = np.asarray(inputs["in_conv_w"], np.float32).reshape(CH, C)
    convs = [(np.asarray(inputs[f"conv{k}_w"], np.float32),
              np.asarray(inputs[f"conv{k}_b"], np.float32))
             for k in (1, 3, 5, 7)]
    # s'[g, j] = -scales3[g, 3-j]
    sadj = -np.asarray(inputs["scales3"], np.float32)[:, ::-1]    # (64, 4)
    sadj4 = np.repeat(sadj[:, None, :], 4, axis=1)                # (64, 4, 4)
    bn_scale = (np.asarray(inputs["bn_gamma"]) /
                np.sqrt(np.asarray(inputs["bn_var"]) + 1e-5)).astype(np.float32)
    bn_bias = (np.asarray(inputs["bn_beta"]) -
               np.asarray(inputs["bn_mean"]) * bn_scale).astype(np.float32)
    wts = (w_in, np.asarray(inputs["in_conv_b"], np.float32), convs, sadj4,
           np.asarray(inputs["bc_w"], np.float32).reshape(CH, CH),
           bn_scale, bn_bias,
           np.asarray(inputs["fc_w"], np.float32).reshape(CH),
           float(np.asarray(inputs["fc_b"])[0]))
    in_maps = []
    for core in range(8):
        b, half = core // 2, core % 2
        in_maps.append(_prep_core_inputs(cen[b], half == 1, wts))
    return in_maps


def kernel(**inputs):
    in_maps = make_in_maps(inputs)
    nc = _get_nc()
    res = bass_utils.run_bass_kernel_spmd(nc, in_maps,
                                          core_ids=list(range(8)))
    out = np.empty((4, C, 160, W), np.float32)
    for core in range(8):
        b, half = core // 2, core % 2
        o = np.concatenate(
            [np.asarray(res.results[core]["out0"]).astype(np.float32)
             .reshape(128, 2 * OWNH, W),
             np.asarray(res.results[core]["out1"]).astype(np.float32)
             .reshape(128, 2 * OWNH, W)], 0)
        # rows: [A(40) | B(40)] in flipped-shard coords
        if half == 1:
            o = o[:, ::-1]
        out[b, :, half * 80:(half + 1) * 80] = o
    return out


# revision 11
# speedup vs baseline: 1.0778x; 1.0778x over previous
"""Trainium2 Bass kernel for nn_ExpansionContrastModule.

Sharding: 8 cores = 4 batches x 2 H-halves (80 rows each). Bottom halves are
row-flipped on the host (conv weights H-flipped to match) so that image-pad
rows always sit at shard-top; the dwconv-product-sort stage is flip-invariant
because it only depends on the +/-v_m offset pairs and the sort is
permutation-invariant over directions.

Within a core the 80 owned rows split into two 40-row sub-halves A/B mapped to
SBUF partitions 0:64 / 64:128 (64 channels each), computed in lockstep:
conv matmuls use block-diagonal duplicated weights (K=128, M=128) and all
vector ops run fully packed [128, N].

Restructured dwconv-contrast: with o1_m(p) = x(p) - x(p + v_m),
  o_m(p) = o1_m(p) * (x(p) - x(p - v_m)) = -o1_m(p) * o1_m(p - v_m),
so only 4 difference maps per branch are needed; the negation is absorbed by
using adjusted scales s'[g, j] = -scales3[g, 3-j] on the sorted products
t_m = o1_m(p) * o1_m(p - v_m).

Geometry (per half, local coords): shard = 100 rows (10 pad/halo + 80 + 10
halo). Owned rows = shard 10..89; A owns 10..49, B owns 50..89.
x0 (in_conv out): 60 rows per half, row r <-> shard r (A) / 40+r (B).
x_k (branch conv out): rows_x = 40+2d rows, row i <-> shard 10-d+i (A),
50-d+i (B); width Wx = 160+4d with real cols at [2d, 2d+160).
Shard rows 0..9 are always image-pad (flip trick) -> x0 rows 0..9 and
x_k rows 0..d-1 of the A half are memset to zero.

v2 changes vs baseline:
- all DMAs on HWDGE (nc.sync) instead of SWDGE (gpsimd) - SWDGE descriptor
  generation starves while DVE holds the shared SBUF port.
- phase A loads cen in big chunks instead of per-row-tile.
- final stage computes the z-chain once (it is shared between the two mask
  halves; baseline computed it twice) at 20-row granularity, with bf16
  outputs (host upcasts; tolerance is 2e-2, bf16 rounding adds ~4e-4).
"""

import os

os.environ.setdefault("MYCRO_LOCAL_CACHE", "1")

import numpy as np
import ml_dtypes

import concourse.bass as bass
import concourse.bacc as bacc
import concourse.mybir as mybir
from concourse.tile import TileContext
from concourse import bass_utils

W = 160
SH = 100          # shard rows
HALO = 10
OWNH = 40         # owned rows per half
C = 256
CH = 64           # trunk channels
RB = 10           # contrast-stage block rows
KS = [1, 3, 5, 7]         # branch conv kernel sizes
DIL = [1, 3, 5, 7]        # branch dwconv dilations (= shift)
X0R = 60
WP0 = W + 6       # x0 width (bf16 path), real cols at [3, 163)
RT = 3            # conv rows per psum tile (3*160 = 480 <= 512)
FCH = 20          # final-stage chunk rows

# fp8 DoubleRow branch-conv path: flattened rows of width 192 (step%16==0),
# real cols at [16, 176). Taps are paired vertically ((ki,kj),(ki+1,kj)) so
# the two K-subtiles of one DR matmul read x0f at +192 elements.
W8 = 192
CL = 16           # real-col origin in width-192 layout
X8R = X0R + 1     # one spare zero row: zero-weight second subtile of
                  # single-tap DR matmuls reads one row past the last
SX = 16.0         # fp8 scale on x0
SWT = 64.0        # fp8 scale on branch conv weights

F32 = mybir.dt.float32
BF16 = mybir.dt.bfloat16
F8 = mybir.dt.float8e4
ALU = mybir.AluOpType
ACTF = mybir.ActivationFunctionType


def dr_pairs(ksz):
    """Vertically paired taps for DoubleRow; None second = zero weights."""
    out = []
    for kj in range(ksz):
        ki = 0
        while ki + 1 < ksz:
            out.append(((ki, kj), (ki + 1, kj)))
            ki += 2
        if ki < ksz:
            out.append(((ki, kj), None))
    return out


SLOTS = [len(dr_pairs(k)) for k in KS]            # [1, 6, 15, 28]
SLOT_OFF = [sum(SLOTS[:i]) for i in range(4)]     # [0, 1, 7, 22]
NSLOT = sum(SLOTS)                                # 50


def build_nc():
    nc = bacc.Bacc("TRN2", target_bir_lowering=False, debug=False,
                   enable_asserts=False, num_devices=8)

    def dram(name, shape, dt, kind="ExternalInput"):
        return nc.dram_tensor(name, list(shape), dt, kind=kind).ap()

    cenb = [dram(f"cenb{c}", (128, SH * W), BF16) for c in range(2)]
    win = dram("win", (128, 128), BF16)
    wtap = dram("wtap", (128, NSLOT * 256), F8)
    wbc = dram("wbc", (128, 128), BF16)
    wfc = dram("wfc", (128, 2), BF16)
    ones1 = dram("ones1", (1, 128), BF16)
    bin_ = dram("bin", (128, 1), F32)
    cb = dram("cb", (128, 4), F32)
    sadj = dram("sadj", (128, 16), F32)
    bnsc = dram("bnsc", (128, 1), F32)
    bnbi = dram("bnbi", (128, 1), F32)
    fcb = dram("fcb", (1, 1), F32)
    outs = [dram(f"out{c}", (128, OWNH * 2 * W), BF16, kind="ExternalOutput")
            for c in range(2)]

    with TileContext(nc) as tc:
        with tc.tile_pool(name="cpool", bufs=1) as cp, \
             tc.tile_pool(name="inpool", bufs=2) as ip, \
             tc.tile_pool(name="x0pool", bufs=1) as x0p, \
             tc.tile_pool(name="xpool", bufs=2) as xp, \
             tc.tile_pool(name="o1pool", bufs=1) as o1p, \
             tc.tile_pool(name="tpool", bufs=1) as tp, \
             tc.tile_pool(name="ypool", bufs=2) as yp, \
             tc.tile_pool(name="vpool", bufs=1) as vp, \
             tc.tile_pool(name="pspool", bufs=1, space="PSUM") as pp:

            # ---- constants to SBUF
            win_s = cp.tile_from(win, name="win_s")
            wtap_s = cp.tile_from(wtap, name="wtap_s")
            wbc_s = cp.tile_from(wbc, name="wbc_s")
            wfc_s = cp.tile_from(wfc, name="wfc_s")
            ones_s = cp.tile_from(ones1, name="ones_s")
            bin_s = cp.tile_from(bin_, name="bin_s")
            cb_s = cp.tile_from(cb, name="cb_s")
            sadj_s = cp.tile_from(sadj, name="sadj_s")
            bnsc_s = cp.tile_from(bnsc, name="bnsc_s")
            bnbi_s = cp.tile_from(bnbi, name="bnbi_s")
            fcb_s = cp.tile_from(fcb, name="fcb_s")

            # ---- Phase A: in_conv -> x0f [128, 61*192] fp8 (values * SX)
            # cen chunks: per (h, chunk of 15 x0-rows, c) load [128, 15*160].
            CKR = 15
            x0 = x0p.tile([128, X8R * W8], F8, name="x0")
            nc.gpsimd.memset(x0[:, :], 0.0)
            x0v = x0[:, 0:X0R * W8].rearrange("p (r w) -> p r w", w=W8)
            for h, base in ((0, 0), (1, 40)):
                for ck in range(0, X0R, CKR):
                    cts = []
                    for c in range(2):
                        ct = ip.tile([128, FCH * W], BF16, tag=f"cin{c}",
                                     name=f"ct{c}")
                        nc.sync.dma_start(
                            out=ct[:, 0:CKR * W],
                            in_=cenb[c][:, (base + ck) * W:(base + ck + CKR) * W])
                        cts.append(ct)
                    for t0 in range(ck, ck + CKR, RT):
                        n = RT * W
                        ps = pp.tile([128, n], F32, tag="cvps", bufs=3,
                                     name="ps_in")
                        for c in range(2):
                            nc.tensor.matmul(
                                ps[h * 64:h * 64 + 64, :],
                                lhsT=win_s[:, c * 64:c * 64 + 64],
                                rhs=cts[c][:, (t0 - ck) * W:(t0 - ck) * W + n],
                                start=(c == 0), stop=(c == 1))
                        nc.scalar.activation(
                            x0v[h * 64:h * 64 + 64, t0:t0 + RT, CL:CL + W],
                            ps[h * 64:h * 64 + 64, :]
                            .rearrange("p (r w) -> p r w", w=W),
                            ACTF.Identity, bias=bin_s[h * 64:h * 64 + 64, 0:1],
                            scale=SX)
            # zero image-pad rows of A half (shard rows 0..9)
            nc.gpsimd.memset(x0[0:64, 0:HALO * W8], 0.0)

            # ---- vmax / vsum accumulators [128, 40*160] bf16
            vmax = vp.tile([128, OWNH * W], BF16, name="vmax")
            vsum = vp.tile([128, OWNH * W], BF16, name="vsum")

            from concourse.ap import AP as _AP
            x0flat = x0[:, :]
            pdim = list(x0flat.ap[0])  # [step, 128] partition dim

            for k in range(4):
                d = DIL[k]
                ksz = KS[k]
                pad = ksz // 2
                rows_x = OWNH + 2 * d
                pairs = dr_pairs(ksz)
                xk = xp.tile([128, rows_x * W8], BF16, tag="x", name=f"x{k}")
                xkf = xk[:, :]
                xv = xkf.rearrange("p (r w) -> p r w", w=W8)

                # branch conv over the flattened span [CL, (rows_x-1)*192+CL+W)
                QE = (rows_x - 1) * W8 + CL + W
                for q0 in range(CL, QE, 512):
                    nq = min(512, QE - q0)
                    ps = pp.tile([128, 512], F32, tag="cvps", bufs=3,
                                 name=f"ps{k}")
                    for si, (tap1, tap2) in enumerate(pairs):
                        dy1, dx1 = tap1[0] - pad, tap1[1] - pad
                        slot = SLOT_OFF[k] + si
                        delta = (HALO - d + dy1) * W8 + dx1
                        lhsT = wtap_s[:, slot * 256:slot * 256 + 256] \
                            .rearrange("p (a m) -> p a m", a=2)
                        rhs = _AP(x0flat.tensor, q0 + delta,
                                  [pdim, [W8, 2], [1, nq]])
                        nc.tensor.matmul(
                            ps[:, 0:nq], lhsT=lhsT, rhs=rhs,
                            start=(si == 0), stop=(si == len(pairs) - 1),
                            perf_mode=mybir.MatmulPerfMode.DoubleRow)
                    nc.scalar.activation(
                        xkf[:, q0:q0 + nq], ps[:, 0:nq],
                        ACTF.Identity, bias=cb_s[:, k:k + 1],
                        scale=1.0 / (SX * SWT))
                # zero col pads (conv wrote garbage there): row0 left, the
                # right+left strip between rows, last row right.
                nc.gpsimd.memset(xk[:, 0:CL], 0.0)
                nc.gpsimd.memset(
                    xk[:, CL + W:CL + W + (rows_x - 1) * W8]
                    .rearrange("p (r w) -> p r w", w=W8)[:, :, 0:2 * CL], 0.0)
                nc.gpsimd.memset(
                    xk[:, (rows_x - 1) * W8 + CL + W:rows_x * W8], 0.0)
                # zero image-pad rows of A half: x rows 0..d-1
                nc.gpsimd.memset(xk[0:64, 0:d * W8], 0.0)

                # ---- contrast stage: blocks of RB owned rows
                WPK = W + 2 * d
                vs = [(-d, -d), (-d, 0), (-d, d), (0, -d)]
                for b in range(0, OWNH, RB):
                    ts = []
                    for j, (dy, dx) in enumerate(vs):
                        o1 = o1p.tile([128, (RB + d) * WPK], BF16, tag=f"o1_{j}",
                                      name=f"o1_{k}_{b}_{j}")
                        o1v = o1[:, :].rearrange("p (r w) -> p r w", w=WPK)
                        xr0 = d + b   # x row of o1 row 0
                        nc.vector.tensor_sub(
                            o1v[:, :, :],
                            xv[:, xr0:xr0 + RB + d, CL - d:CL - d + WPK],
                            xv[:, xr0 + dy:xr0 + dy + RB + d,
                               CL - d + dx:CL - d + dx + WPK])
                        tj = tp.tile([128, RB * W], BF16, tag=f"t{j}",
                                     name=f"t{k}_{b}_{j}")
                        nc.vector.tensor_mul(
                            tj[:, :].rearrange("p (r w) -> p r w", w=W),
                            o1v[:, 0:RB, d:d + W],
                            o1v[:, -dy:-dy + RB, d - dx:d - dx + W])
                        ts.append(tj)
                    t0_, t1_, t2_, t3_ = [t[:, :] for t in ts]
                    e1 = tp.tile([128, RB * W], BF16, tag="e1",
                                 name=f"e{k}_{b}")[:, :]
                    # 5-comparator sort network (ascending finals:
                    # t3_=o(1), t1_=o(2), t0_=o(3), e1=o(4))
                    nc.vector.tensor_tensor(e1, t0_, t1_, ALU.max)
                    nc.vector.tensor_tensor(t0_, t0_, t1_, ALU.min)
                    nc.vector.tensor_tensor(t1_, t2_, t3_, ALU.max)
                    nc.vector.tensor_tensor(t2_, t2_, t3_, ALU.min)
                    nc.vector.tensor_tensor(t3_, t0_, t2_, ALU.min)
                    nc.vector.tensor_tensor(t0_, t0_, t2_, ALU.max)
                    nc.vector.tensor_tensor(t2_, e1, t1_, ALU.min)
                    nc.vector.tensor_tensor(e1, e1, t1_, ALU.max)
                    nc.vector.tensor_tensor(t1_, t0_, t2_, ALU.min)
                    nc.vector.tensor_tensor(t0_, t0_, t2_, ALU.max)
                    # weighted sum with adjusted scales
                    y = yp.tile([128, RB * W], BF16, tag="y",
                                name=f"y{k}_{b}")[:, :]
                    nc.vector.tensor_scalar_mul(y, t3_, sadj_s[:, 4 * k:4 * k + 1])
                    nc.vector.scalar_tensor_tensor(
                        y, t1_, sadj_s[:, 4 * k + 1:4 * k + 2], y,
                        op0=ALU.mult, op1=ALU.add)
                    nc.vector.scalar_tensor_tensor(
                        y, t0_, sadj_s[:, 4 * k + 2:4 * k + 3], y,
                        op0=ALU.mult, op1=ALU.add)
                    nc.vector.scalar_tensor_tensor(
                        y, e1, sadj_s[:, 4 * k + 3:4 * k + 4], y,
                        op0=ALU.mult, op1=ALU.add)
                    vmx = vmax[:, b * W:(b + RB) * W]
                    vsm = vsum[:, b * W:(b + RB) * W]
                    if k == 0:
                        nc.vector.tensor_copy(vmx, y)
                        nc.vector.tensor_copy(vsm, y)
                    else:
                        nc.vector.tensor_tensor(vmx, vmx, y, ALU.max)
                        nc.vector.tensor_tensor(vsm, vsm, y, ALU.add)

            # ---- final stage: chunks of FCH rows (columns shared by both
            # halves; z-chain computed once, mask extracted per half).
            for f0 in range(0, OWNH, FCH):
                n = FCH * W                       # 3200
                sl = slice(f0 * W, (f0 + FCH) * W)
                mt = o1p.tile([128, n], BF16, tag="o1_0", name="mt")[:, :]
                nc.vector.scalar_tensor_tensor(
                    mt, vsum[:, sl], 0.25, vmax[:, sl],
                    op0=ALU.mult, op1=ALU.add)
                mr = o1p.tile([128, n], BF16, tag="o1_1", name="mr")[:, :]
                nc.scalar.activation(mr, mt, ACTF.Relu)
                zlin = o1p.tile([128, n], BF16, tag="o1_2", name="zlin")[:, :]
                zsig = o1p.tile([128, n], BF16, tag="o1_3", name="zsig")[:, :]
                NSUB = (n + 479) // 480
                for s in range(NSUB):
                    c0, c1 = s * 480, min(n, s * 480 + 480)
                    zps = pp.tile([128, 480], F32, tag="zps", bufs=2,
                                  name="zps")
                    nc.tensor.matmul(zps[:, 0:c1 - c0], lhsT=wbc_s[:, :],
                                     rhs=mr[:, c0:c1], start=True, stop=True)
                    nc.scalar.activation(zlin[:, c0:c1], zps[:, 0:c1 - c0],
                                         ACTF.Identity, bias=bnbi_s[:, 0:1],
                                         scale=bnsc_s[:, 0:1])
                    nc.scalar.activation(zsig[:, c0:c1], zps[:, 0:c1 - c0],
                                         ACTF.Sigmoid, bias=bnbi_s[:, 0:1],
                                         scale=bnsc_s[:, 0:1])
                zt = tp.tile([128, n], BF16, tag="t0", bufs=1, name="zt")[:, :]
                nc.vector.tensor_mul(zt, zlin, zsig)
                for h in range(2):
                    msk = tp.tile([1, n], BF16, tag="msk", bufs=1,
                                  name="msk")
                    mbs = tp.tile([128, n], BF16, tag="mbs", bufs=1,
                                  name="mbs")[:, :]
                    for s in range(NSUB):
                        c0, c1 = s * 480, min(n, s * 480 + 480)
                        lps = pp.tile([1, 480], F32, tag="lps", bufs=1,
                                      name="lps")
                        nc.tensor.matmul(lps[:, 0:c1 - c0],
                                         lhsT=wfc_s[:, h:h + 1],
                                         rhs=zt[:, c0:c1],
                                         start=True, stop=True)
                        nc.scalar.activation(msk[:, c0:c1], lps[:, 0:c1 - c0],
                                             ACTF.Sigmoid, bias=fcb_s[0:1, 0:1])
                        mb = pp.tile([128, 480], F32, tag="mb", bufs=1,
                                     name="mb")
                        nc.tensor.matmul(mb[:, 0:c1 - c0], lhsT=ones_s[:, :],
                                         rhs=msk[:, c0:c1], start=True,
                                         stop=True)
                        nc.scalar.activation(mbs[:, c0:c1], mb[:, 0:c1 - c0],
                                             ACTF.Identity)
                    for c in range(2):
                        cent = ip.tile([128, FCH * W], BF16, tag=f"cin{c}",
                                       name="cent")
                        src = (HALO + h * OWNH + f0) * W
                        nc.sync.dma_start(out=cent[:, :],
                                          in_=cenb[c][:, src:src + n])
                        ot = yp.tile([128, n], BF16, tag=f"ot{c}", bufs=1,
                                     name="ot")
                        nc.vector.scalar_tensor_tensor(
                            ot[:, :], mbs, 1.0, cent[:, :],
                            op0=ALU.add, op1=ALU.mult)
                        dst = (h * OWNH + f0) * W
                        nc.sync.dma_start(out=outs[c][:, dst:dst + n],
                                          in_=ot[:, :])
    nc.compile()
    nc.finalize()
    return nc


_NC_CACHE = None


def _get_nc():
    global _NC_CACHE
    if _NC_CACHE is None:
        _NC_CACHE = build_nc()
    return _NC_CACHE


def _prep_core_inputs(cen_b, flip, wts):
    """cen_b: (256, 160, 160) fp32 for this batch; flip: bottom half?"""
    (w_in, b_in, convs, scales_adj, bc_w, bn_scale, bn_bias,
     fc_w, fc_b) = wts
    lo = (1 if flip else 0) * 80 - HALO
    sh = np.zeros((C, SH, W), np.float32)
    r0, r1 = max(0, lo), min(160, lo + SH)
    sh[:, r0 - lo:r1 - lo] = cen_b[:, r0:r1]
    if flip:
        sh = sh[:, ::-1]
    sh = np.ascontiguousarray(sh)

    bf = ml_dtypes.bfloat16
    f8 = ml_dtypes.float8_e4m3
    wtap = np.zeros((128, NSLOT * 256), f8)

    def put_tap(slot, sub, blk):
        base = slot * 256 + sub * 128
        wtap[0:64, base:base + 64] = blk
        wtap[64:128, base + 64:base + 128] = blk

    for k in range(4):
        ksz = KS[k]
        cw = convs[k][0]
        if flip:
            cw = cw[:, :, ::-1, :]
        for si, (tap1, tap2) in enumerate(dr_pairs(ksz)):
            slot = SLOT_OFF[k] + si
            for sub, tap in ((0, tap1), (1, tap2)):
                if tap is None:
                    continue
                blk = (cw[:, :, tap[0], tap[1]].T * SWT).astype(f8)  # [ci, co]
                put_tap(slot, sub, blk)

    win = np.zeros((128, 128), bf)
    win[:, 0:64] = w_in[:, 0:128].T.astype(bf)
    win[:, 64:128] = w_in[:, 128:256].T.astype(bf)

    wbc = np.zeros((128, 128), bf)
    wbc[0:64, 0:64] = bc_w.T.astype(bf)
    wbc[64:128, 64:128] = bc_w.T.astype(bf)

    wfc = np.zeros((128, 2), bf)
    wfc[0:64, 0] = fc_w.astype(bf)
    wfc[64:128, 1] = fc_w.astype(bf)

    dup = lambda v: np.concatenate([v, v]).astype(np.float32).reshape(128, -1)
    m = {
        "cenb0": sh[0:128].reshape(128, SH * W).astype(bf),
        "cenb1": sh[128:256].reshape(128, SH * W).astype(bf),
        "win": win,
        "wtap": wtap,
        "wbc": wbc,
        "wfc": wfc,
        "ones1": np.ones((1, 128), bf),
        "bin": dup(b_in * SX),
        "cb": np.concatenate([np.stack([cb for _, cb in convs], 1)] * 2, 0)
              .astype(np.float32),
        "sadj": np.concatenate([scales_adj.reshape(64, 16)] * 2, 0)
                .astype(np.float32),
        "bnsc": dup(bn_scale),
        "bnbi": dup(bn_bias),
        "fcb": np.full((1, 1), fc_b, np.float32),
    }
    return m


def make_in_maps(inputs):
    cen = np.asarray(inputs["cen"], np.float32)
    w_in = np.asarray(inputs["in_conv_w"], np.float32).reshape(CH, C)
    convs = [(np.asarray(inputs[f"conv{k}_w"], np.float32),
              np.asarray(inputs[f"conv{k}_b"], np.float32))
             for k in (1, 3, 5, 7)]
    # s'[g, j] = -scales3[g, 3-j]
    sadj = -np.asarray(inputs["scales3"], np.float32)[:, ::-1]    # (64, 4)
    sadj4 = np.repeat(sadj[:, None, :], 4, axis=1)                # (64, 4, 4)
    bn_scale = (np.asarray(inputs["bn_gamma"]) /
                np.sqrt(np.asarray(inputs["bn_var"]) + 1e-5)).astype(np.float32)
    bn_bias = (np.asarray(inputs["bn_beta"]) -
               np.asarray(inputs["bn_mean"]) * bn_scale).astype(np.float32)
    wts = (w_in, np.asarray(inputs["in_conv_b"], np.float32), convs, sadj4,
           np.asarray(inputs["bc_w"], np.float32).reshape(CH, CH),
           bn_scale, bn_bias,
           np.asarray(inputs["fc_w"], np.float32).reshape(CH),
           float(np.asarray(inputs["fc_b"])[0]))
    in_maps = []
    for core in range(8):
        b, half = core // 2, core % 2
        in_maps.append(_prep_core_inputs(cen[b], half == 1, wts))
    return in_maps


def kernel(**inputs):
    in_maps = make_in_maps(inputs)
    nc = _get_nc()
    res = bass_utils.run_bass_kernel_spmd(nc, in_maps,
                                          core_ids=list(range(8)))
    out = np.empty((4, C, 160, W), np.float32)
    for core in range(8):
        b, half = core // 2, core % 2
        o = np.concatenate(
            [np.asarray(res.results[core]["out0"]).astype(np.float32)
             .reshape(128, 2 * OWNH, W),
             np.asarray(res.results[core]["out1"]).astype(np.float32)
             .reshape(128, 2 * OWNH, W)], 0)
        # rows: [A(40) | B(40)] in flipped-shard coords
        if half == 1:
            o = o[:, ::-1]
        out[b, :, half * 80:(half + 1) * 80] = o
    return out
